# revision 5
# baseline (speedup 1.0000x reference)
"""GNN message-passing kernel for Trainium2 (8 NeuronCores, SPMD).

Strategy (edge-parallel by destination):
  * Host sorts edges by destination node, assigns 128-node blocks to
    (core, window-slot) pairs so per-slot edge counts are balanced across
    cores (one shared compile-time tile schedule for all 8 cores).
  * Host pre-gathers x[row] / edge_attr per edge, folds the per-edge
    scale wrc = wts / max(count[col], 1) directly into the gathered
    features (relu(w*h) == w*relu(h) for w >= 0), and folds mw2 @ uw1r
    into a single weight Wc so the hid-basis scatter feeds the update
    MLP without a per-window mw2 matmul. ub2 is added on the host.
  * All heavy DMA goes through the gpsimd software DGE so packets spread
    across all 16 DMA engines (HWDGE queues pin to a single engine).
    cw is SBUF-resident; eax streams in large groups; xcon streams one
    chunk per window-quad so it never competes with eax at startup.
  * Device, per 128-edge tile: h = eax.T @ mw1_aug (PSUM), relu on ACT
    (one per 8 tiles), one-hot S built on DVE one instr per 4 tiles in
    an interleaved [node, tile] layout (keeps operands packed for DVE
    fast modes), scatter T_w[hid, node] += G.T @ S per 128-node window.
  * Update MLP per 4-window quad: h2 = uw1aug.T @ xcon + Wc.T @ T,
    relu, out = uw2.T @ h2r, written bf16.
  * All matmuls bf16: measured on TRN2, fp8 matmuls are not faster per
    column than bf16 (and fp8 DoubleRow is slower), so bf16 minimizes
    PE time while halving DMA vs f32.
"""
import numpy as np
import ml_dtypes

import concourse.bacc as bacc
import concourse.tile as tile
from concourse import mybir
from concourse.bass_utils import run_bass_kernel_spmd

BF = mybir.dt.bfloat16
F32 = mybir.dt.float32
bf16 = ml_dtypes.bfloat16

P = 128
NCORES = 8
HID = 128
NODE_D = 64
EDGE_D = 32
GLOB_D = 32
FEAT = 97                    # x*w | ea*w | w
XCON_R = 98                  # x | u | s | ones
GROUP = 64                   # 128-edge tiles per eax DMA group
SGRP = 4                     # tiles per S-build batch
RGRP = 8                     # tiles per relu batch (PSUM group)
QUAD = 4                     # windows per update-MLP batch
OGRP = 2                     # quads per output DMA

# const blob column layout (bf16)
_B_MW1 = 0                   # [0:97, 0:128]      mw1_aug (w/ mb1 row)
_B_UW1 = 128                 # [0:98, 128:256]    uw1aug (x|u|v_row|ub1)
_B_WC = 256                  # [0:128, 256:384]   Wc = mw2 @ uw1r
_B_UW2 = 384                 # [0:128, 384:448]   uw2
_B_IR = (448, 576, 832, 1216)  # iotaRep nk=1..4: col c -> c // nk
BLOB_W = 1728

CFG = {
    "geax": 4, "gs": 3, "gg": 3, "gn": 2, "go": 2,
    "ph": 2, "pt": 2, "p2": 1,
    "h2r_act": True,         # h2r relu on ACT (else DVE)
    "hh_act": False,         # hh copy on ACT (else DVE)
}

_program_cache: dict = {}
_last_results = None


def _build_program(t_sched):
    nt = sum(t_sched)
    e_pad = nt * P
    nslots = len(t_sched)
    nsh = nslots * P
    nquads = -(-nslots // QUAD)

    nc = bacc.Bacc()
    eax_d = nc.dram_tensor("eax", [FEAT, e_pad], BF, kind="ExternalInput")
    cwv_d = nc.dram_tensor("cwv", [P, nt], BF, kind="ExternalInput")
    blob_d = nc.dram_tensor("blob", [P, BLOB_W], BF, kind="ExternalInput")
    xcon_d = nc.dram_tensor("xcon", [XCON_R, nsh], BF, kind="ExternalInput")
    out_d = nc.dram_tensor("out", [64, nsh], BF, kind="ExternalOutput")

    with tile.TileContext(nc) as tc:
        with (
            tc.tile_pool(name="consts", bufs=1) as consts,
            tc.tile_pool(name="geax", bufs=CFG["geax"]) as geax,
            tc.tile_pool(name="gs", bufs=CFG["gs"]) as gs,
            tc.tile_pool(name="gg", bufs=CFG["gg"]) as gg,
            tc.tile_pool(name="gn", bufs=CFG["gn"]) as gn,
            tc.tile_pool(name="go", bufs=CFG["go"]) as go,
            tc.tile_pool(name="ph", bufs=CFG["ph"], space="PSUM") as ph,
            tc.tile_pool(name="pt", bufs=CFG["pt"], space="PSUM") as pt,
            tc.tile_pool(name="p2", bufs=CFG["p2"], space="PSUM") as p2,
            tc.tile_pool(name="po", bufs=1, space="PSUM") as po,
        ):
            blob_t = consts.tile([P, BLOB_W], BF)
            nc.gpsimd.dma_start(blob_t[:], blob_d[:])
            mw1_t = blob_t[0:FEAT, _B_MW1:_B_MW1 + HID]
            uw1_t = blob_t[0:XCON_R, _B_UW1:_B_UW1 + HID]
            wc_t = blob_t[0:HID, _B_WC:_B_WC + HID]
            uw2_t = blob_t[0:HID, _B_UW2:_B_UW2 + 64]
            cwv_t = consts.tile([P, nt], BF)
            nc.gpsimd.dma_start(cwv_t[:], cwv_d[:])
            xcon_t = consts.tile([XCON_R, nsh], BF)

            def load_xcon_quad(qi):
                if qi < nquads:
                    c0 = qi * QUAD * P
                    c1 = min(nsh, (qi + 1) * QUAD * P)
                    nc.gpsimd.dma_start(
                        xcon_t[:, c0:c1], xcon_d[:, c0:c1])

            load_xcon_quad(0)

            eax_g = None
            ptq = None
            o_sb = None
            t = 0
            for j in range(nslots):
                tj = t_sched[j]
                q, jr = divmod(j, QUAD)
                if jr == 0:
                    qw = min(QUAD, nslots - q * QUAD)
                    ptq = pt.tile([P, QUAD * P], F32, space="PSUM")
                    load_xcon_quad(q + 1)
                rdone = 0
                while rdone < tj:
                    nr = min(RGRP, tj - rdone)
                    h8_ps = ph.tile([P, RGRP * HID], F32, space="PSUM")
                    schunks = []
                    kdone = 0
                    while kdone < nr:
                        nk = min(SGRP, nr - kdone)
                        for i in range(kdone, kdone + nk):
                            g, r = divmod(t, GROUP)
                            if r == 0:
                                n = min(GROUP, nt - g * GROUP)
                                eax_g = geax.tile(
                                    [FEAT, GROUP * P], BF, tag="eax")
                                nc.gpsimd.dma_start(
                                    eax_g[:, 0:n * P],
                                    eax_d[:, g * GROUP * P:
                                          (g * GROUP + n) * P],
                                )
                            nc.tensor.matmul(
                                h8_ps[:, i * HID:(i + 1) * HID],
                                lhsT=eax_g[:, r * P:(r + 1) * P],
                                rhs=mw1_t,
                                start=True, stop=True,
                            )
                            t += 1
                        s4 = gs.tile([P, SGRP * P], BF, tag="S")
                        irc = _B_IR[nk - 1]
                        nc.vector.tensor_tensor(
                            s4[:, 0:nk * P].rearrange(
                                "p (n t) -> p n t", t=nk),
                            blob_t[:, irc:irc + nk * P].rearrange(
                                "p (n t) -> p n t", t=nk),
                            cwv_t[:, t - nk:t, None]
                            .rearrange("p t o -> p o t")
                            .broadcast_to([P, P, nk]),
                            op=mybir.AluOpType.is_equal,
                        )
                        schunks.append((kdone, nk, s4))
                        kdone += nk
                    g8 = gg.tile([P, RGRP * HID], BF, tag="G")
                    nc.scalar.activation(
                        g8[:, 0:nr * HID], h8_ps[:, 0:nr * HID],
                        mybir.ActivationFunctionType.Relu,
                    )
                    for k0, nk, s4 in schunks:
                        s4v = s4[:, 0:nk * P].rearrange(
                            "p (n t) -> p n t", t=nk)
                        for i in range(nk):
                            ti = rdone + k0 + i
                            nc.tensor.matmul(
                                ptq[:, jr * P:(jr + 1) * P],
                                lhsT=g8[:, (k0 + i) * HID:(k0 + i + 1) * HID],
                                rhs=s4v[:, :, i],
                                start=(ti == 0), stop=(ti == tj - 1),
                            )
                    rdone += nr
                # quad boundary: run update MLP on 4 windows at once
                if jr == qw - 1:
                    w = qw * P
                    n0 = q * QUAD * P
                    hh4 = gn.tile([HID, QUAD * P], BF, tag="hh")
                    if CFG["hh_act"]:
                        nc.scalar.copy(hh4[:, 0:w], ptq[:, 0:w])
                    else:
                        nc.vector.tensor_copy(hh4[:, 0:w], ptq[:, 0:w])
                    h2_ps = p2.tile([HID, QUAD * P], F32, space="PSUM")
                    nc.tensor.matmul(
                        h2_ps[:, 0:w], lhsT=uw1_t,
                        rhs=xcon_t[:, n0:n0 + w],
                        start=True, stop=False,
                    )
                    nc.tensor.matmul(
                        h2_ps[:, 0:w], lhsT=wc_t, rhs=hh4[:, 0:w],
                        start=False, stop=True,
                    )
                    h2r = gn.tile([HID, QUAD * P], BF, tag="h2r")
                    if CFG["h2r_act"]:
                        nc.scalar.activation(
                            h2r[:, 0:w], h2_ps[:, 0:w],
                            mybir.ActivationFunctionType.Relu,
                        )
                    else:
                        nc.vector.tensor_scalar_max(
                            h2r[:, 0:w], h2_ps[:, 0:w], 0.0)
                    o_ps = po.tile([64, QUAD * P], F32, space="PSUM")
                    nc.tensor.matmul(o_ps[:, 0:w], lhsT=uw2_t,
                                     rhs=h2r[:, 0:w], start=True, stop=True)
                    qo = q % OGRP
                    if qo == 0:
                        o_sb = go.tile([64, OGRP * QUAD * P], BF, tag="o")
                    nc.vector.tensor_copy(
                        o_sb[:, qo * QUAD * P:qo * QUAD * P + w],
                        o_ps[:, 0:w])
                    if qo == OGRP - 1 or j == nslots - 1:
                        ow = (q - qo) * QUAD * P
                        nc.gpsimd.dma_start(
                            out_d[:, ow:n0 + w],
                            o_sb[:, 0:n0 + w - ow],
                        )
    nc.finalize()
    return nc


def _schedule(col, n_nodes):
    """Assign 128-node blocks to (core, slot) and derive the shared
    per-slot tile schedule."""
    nblk = -(-n_nodes // P)
    nslots = -(-nblk // NCORES)
    nblk_pad = nslots * NCORES
    nsh = nslots * P

    blk = (col >> 7).astype(np.int64)
    order = np.argsort(blk, kind="stable")
    bc = np.bincount(blk, minlength=nblk_pad)
    bstart = np.zeros(nblk_pad + 1, np.int64)
    np.cumsum(bc, out=bstart[1:])

    sorted_blocks = np.argsort(-bc, kind="stable")
    blk_assign = sorted_blocks.reshape(nslots, NCORES)   # [slot, core]
    grp_max = bc[blk_assign].max(axis=1)
    t_sched = [int(v) for v in np.maximum(1, -(-grp_max // P))]
    return t_sched, blk_assign, order, bc, bstart, nslots, nsh


def kernel(x, edge_index, edge_attr, u, node_batch, wts,
           mw1, mb1, mw2, mb2, uw1, ub1, uw2, ub2):
    x = np.asarray(x, np.float32)
    edge_index = np.asarray(edge_index)
    edge_attr = np.asarray(edge_attr, np.float32)
    u = np.asarray(u, np.float32)
    node_batch = np.asarray(node_batch).astype(np.int64)
    wts = np.asarray(wts, np.float32).reshape(-1)
    mw1 = np.asarray(mw1, np.float32)
    mb1 = np.asarray(mb1, np.float32)
    mw2 = np.asarray(mw2, np.float32)
    mb2 = np.asarray(mb2, np.float32)
    uw1 = np.asarray(uw1, np.float32)
    ub1 = np.asarray(ub1, np.float32)
    uw2 = np.asarray(uw2, np.float32)
    ub2 = np.asarray(ub2, np.float32)

    n_nodes = x.shape[0]
    row = np.asarray(edge_index[0], np.int64)
    col = np.asarray(edge_index[1], np.int64)

    sched = _schedule(col, n_nodes)
    (t_sched, blk_assign, order, bc, bstart, nslots, nsh) = sched
    nt = sum(t_sched)
    e_pad = nt * P

    # per-node stats (host): count, 1/max(cnt,1), weight-sum
    cnt = np.bincount(col, minlength=n_nodes).astype(np.float32)
    rc = 1.0 / np.maximum(cnt, 1.0)
    wsum = np.bincount(col, weights=wts, minlength=n_nodes).astype(np.float32)
    s_node = wsum * rc

    # per-edge
    colof = (col & 127).astype(np.float32)
    wrc = wts * rc[col]

    key = tuple(t_sched)
    if key not in _program_cache:
        _program_cache[key] = _build_program(t_sched)
    nc = _program_cache[key]

    # consts (shared by all cores)
    v_row = mb2 @ uw1[NODE_D:2 * NODE_D, :]              # [HID]
    wc = mw2 @ uw1[NODE_D:2 * NODE_D, :]                 # [HID, HID]
    blob = np.zeros((P, BLOB_W), np.float32)
    blob[0:NODE_D + EDGE_D, _B_MW1:_B_MW1 + HID] = mw1
    blob[NODE_D + EDGE_D, _B_MW1:_B_MW1 + HID] = mb1
    blob[0:NODE_D, _B_UW1:_B_UW1 + HID] = uw1[0:NODE_D, :]
    blob[NODE_D:NODE_D + GLOB_D, _B_UW1:_B_UW1 + HID] = uw1[2 * NODE_D:, :]
    blob[NODE_D + GLOB_D, _B_UW1:_B_UW1 + HID] = v_row
    blob[NODE_D + GLOB_D + 1, _B_UW1:_B_UW1 + HID] = ub1
    blob[0:HID, _B_WC:_B_WC + HID] = wc
    blob[0:HID, _B_UW2:_B_UW2 + 64] = uw2
    for nk in range(1, 5):
        c0 = _B_IR[nk - 1]
        blob[:, c0:c0 + nk * P] = (
            np.arange(nk * P, dtype=np.float32) // nk)[None, :]
    blob_bf = blob.astype(bf16)

    u_per_node = u[node_batch]                           # [N, GLOB_D]

    slot_off = np.zeros(nslots + 1, np.int64)
    np.cumsum(np.asarray(t_sched) * P, out=slot_off[1:])

    in_maps = []
    node_idx_cores = []
    for c in range(NCORES):
        eidx = np.full(e_pad, -1, np.int64)
        nidx = np.full(nsh, -1, np.int64)
        for j in range(nslots):
            b = int(blk_assign[j, c])
            m = int(bc[b])
            o = slot_off[j]
            eidx[o:o + m] = order[bstart[b]:bstart[b] + m]
            n0 = b * P
            nn = min(P, n_nodes - n0)
            if nn > 0:
                nidx[j * P:j * P + nn] = np.arange(n0, n0 + nn)
        evalid = eidx >= 0
        eidxc = np.where(evalid, eidx, 0)
        # eax: [x[row] | edge_attr | 1] * wrc transposed, zeros on pads
        eax = np.empty((e_pad, FEAT), np.float32)
        eax[:, 0:NODE_D] = x[row[eidxc]]
        eax[:, NODE_D:NODE_D + EDGE_D] = edge_attr[eidxc]
        eax[:, NODE_D + EDGE_D] = 1.0
        wcol = np.where(evalid, wrc[eidxc], 0.0).astype(np.float32)
        eax *= wcol[:, None]
        cwv = np.full(e_pad, -1.0, np.float32)
        cwv[evalid] = colof[eidxc[evalid]]

        nvalid = nidx >= 0
        nidxc = np.where(nvalid, nidx, 0)
        xcon = np.zeros((nsh, XCON_R), np.float32)
        xcon[:, 0:NODE_D] = x[nidxc]
        xcon[:, NODE_D:NODE_D + GLOB_D] = u_per_node[nidxc]
        xcon[:, NODE_D + GLOB_D] = s_node[nidxc]
        xcon[:, NODE_D + GLOB_D + 1] = 1.0
        xcon[~nvalid] = 0.0

        in_maps.append({
            "eax": np.ascontiguousarray(eax.T).astype(bf16),
            "cwv": np.ascontiguousarray(
                cwv.reshape(nt, P).T).astype(bf16),
            "blob": blob_bf,
            "xcon": np.ascontiguousarray(xcon.T).astype(bf16),
        })
        node_idx_cores.append((nidx, nvalid))

    res = run_bass_kernel_spmd(nc, in_maps, core_ids=list(range(NCORES)))
    global _last_results
    _last_results = res

    out_full = np.zeros((n_nodes, 64), np.float32)
    for c in range(NCORES):
        nidx, nvalid = node_idx_cores[c]
        oc = np.asarray(res.results[c]["out"], np.float32)   # [64, nsh]
        out_full[nidx[nvalid]] = oc.T[nvalid]
    out_full += ub2[None, :]
    return out_full


# revision 10
# speedup vs baseline: 1.2775x; 1.2775x over previous
"""GNN message-passing kernel for Trainium2 (8 NeuronCores, SPMD).

Strategy (edge-parallel by destination):
  * Host sorts edges by destination node, assigns 128-node blocks to
    (core, window-slot) pairs so per-slot edge counts are balanced across
    cores (one shared compile-time tile schedule for all 8 cores).
  * Host pre-gathers x[row] / edge_attr per edge, folds the per-edge
    scale wrc = wts / max(count[col], 1) directly into the gathered
    features (relu(w*h) == w*relu(h) for w >= 0), and folds mw2 @ uw1r
    into a single weight Wc so the hid-basis scatter feeds the update
    MLP without a per-window mw2 matmul. ub2 is added on the host.
  * All heavy DMA goes through the gpsimd software DGE so packets spread
    across all 16 DMA engines (HWDGE queues pin to a single engine).
    cw is SBUF-resident; eax streams in large groups; xcon streams one
    chunk per window-quad so it never competes with eax at startup.
  * Device, per 128-edge tile: h = eax.T @ mw1_aug (PSUM), relu on ACT
    (one per 8 tiles), one-hot S built on DVE one instr per 4 tiles in
    an interleaved [node, tile] layout (keeps operands packed for DVE
    fast modes), scatter T_w[hid, node] += G.T @ S per 128-node window.
  * Update MLP per 4-window quad: h2 = uw1aug.T @ xcon + Wc.T @ T,
    relu, out = uw2.T @ h2r, written bf16.
  * All matmuls bf16: measured on TRN2, fp8 matmuls are not faster per
    column than bf16 (and fp8 DoubleRow is slower), so bf16 minimizes
    PE time while halving DMA vs f32.
"""
import numpy as np
import ml_dtypes

import concourse.bacc as bacc
import concourse.tile as tile
from concourse import mybir
from concourse.bass_utils import run_bass_kernel_spmd

BF = mybir.dt.bfloat16
F32 = mybir.dt.float32
bf16 = ml_dtypes.bfloat16

P = 128
NCORES = 8
HID = 128
NODE_D = 64
EDGE_D = 32
GLOB_D = 32
FEAT = 97                    # x*w | ea*w | w
XCON_R = 98                  # x | u | s | ones
GROUP = 64                   # 128-edge tiles per eax DMA group
SGRP = 4                     # tiles per S-build batch
RGRP = 4                     # tiles per relu batch (PSUM group)
QUAD = 4                     # windows per update-MLP batch
OGRP = 2                     # quads per output DMA

# const blob column layout (bf16)
_B_MW1 = 0                   # [0:97, 0:128]      mw1_aug (w/ mb1 row)
_B_UW1 = 128                 # [0:98, 128:256]    uw1aug (x|u|v_row|ub1)
_B_WC = 256                  # [0:128, 256:384]   Wc = mw2 @ uw1r
_B_UW2 = 384                 # [0:128, 384:448]   uw2
_B_IR = (448, 576, 832, 1216)  # iotaRep nk=1..4: col c -> c // nk
BLOB_W = 1728

CFG = {
    "geax": 4, "gs": 3, "gg": 3, "gn": 2, "go": 2,
    "ph": 2, "pt": 2, "p2": 2,
    "h2r_act": True,         # h2r relu on ACT (else DVE)
    "hh_act": False,         # hh copy on ACT (else DVE)
}

_program_cache: dict = {}
_last_results = None


def _build_program(t_sched):
    nt = sum(t_sched)
    e_pad = nt * P
    nslots = len(t_sched)
    nsh = nslots * P
    nquads = -(-nslots // QUAD)

    nc = bacc.Bacc()
    eax_d = nc.dram_tensor("eax", [FEAT, e_pad], BF, kind="ExternalInput")
    cwv_d = nc.dram_tensor("cwv", [P, nt], BF, kind="ExternalInput")
    blob_d = nc.dram_tensor("blob", [P, BLOB_W], BF, kind="ExternalInput")
    xcon_d = nc.dram_tensor("xcon", [XCON_R, nsh], BF, kind="ExternalInput")
    out_d = nc.dram_tensor("out", [64, nsh], BF, kind="ExternalOutput")

    with tile.TileContext(nc) as tc:
        with (
            tc.tile_pool(name="consts", bufs=1) as consts,
            tc.tile_pool(name="geax", bufs=CFG["geax"]) as geax,
            tc.tile_pool(name="gs", bufs=CFG["gs"]) as gs,
            tc.tile_pool(name="gg", bufs=CFG["gg"]) as gg,
            tc.tile_pool(name="gn", bufs=CFG["gn"]) as gn,
            tc.tile_pool(name="go", bufs=CFG["go"]) as go,
            tc.tile_pool(name="ph", bufs=CFG["ph"], space="PSUM") as ph,
            tc.tile_pool(name="pt", bufs=CFG["pt"], space="PSUM") as pt,
            tc.tile_pool(name="p2", bufs=CFG["p2"], space="PSUM") as p2,
            tc.tile_pool(name="po", bufs=1, space="PSUM") as po,
        ):
            blob_t = consts.tile([P, BLOB_W], BF)
            nc.gpsimd.dma_start(blob_t[:], blob_d[:])
            mw1_t = blob_t[0:FEAT, _B_MW1:_B_MW1 + HID]
            uw1_t = blob_t[0:XCON_R, _B_UW1:_B_UW1 + HID]
            wc_t = blob_t[0:HID, _B_WC:_B_WC + HID]
            uw2_t = blob_t[0:HID, _B_UW2:_B_UW2 + 64]
            cwv_t = consts.tile([P, nt], BF)
            nc.gpsimd.dma_start(cwv_t[:], cwv_d[:])
            xcon_t = consts.tile([XCON_R, nsh], BF)
            xcon_loaded = False

            eax_g = None
            ptq = None
            o_sb = None
            t = 0
            for j in range(nslots):
                tj = t_sched[j]
                q, jr = divmod(j, QUAD)
                if jr == 0:
                    qw = min(QUAD, nslots - q * QUAD)
                    ptq = pt.tile([P, QUAD * P], F32, space="PSUM")
                rdone = 0
                while rdone < tj:
                    nr = min(RGRP, tj - rdone)
                    h8_ps = ph.tile([P, RGRP * HID], F32, space="PSUM")
                    schunks = []
                    kdone = 0
                    while kdone < nr:
                        nk = min(SGRP, nr - kdone)
                        for i in range(kdone, kdone + nk):
                            g, r = divmod(t, GROUP)
                            if r == 0:
                                n = min(GROUP, nt - g * GROUP)
                                eax_g = geax.tile(
                                    [FEAT, GROUP * P], BF, tag="eax")
                                nc.gpsimd.dma_start(
                                    eax_g[:, 0:n * P],
                                    eax_d[:, g * GROUP * P:
                                          (g * GROUP + n) * P],
                                )
                                if not xcon_loaded:
                                    # behind eax group 0 so the first
                                    # tiles' stream wins the queue race
                                    nc.gpsimd.dma_start(
                                        xcon_t[:], xcon_d[:])
                                    xcon_loaded = True
                            nc.tensor.matmul(
                                h8_ps[:, i * HID:(i + 1) * HID],
                                lhsT=eax_g[:, r * P:(r + 1) * P],
                                rhs=mw1_t,
                                start=True, stop=True,
                            )
                            t += 1
                        s4 = gs.tile([P, SGRP * P], BF, tag="S")
                        irc = _B_IR[nk - 1]
                        nc.vector.tensor_tensor(
                            s4[:, 0:nk * P].rearrange(
                                "p (n t) -> p n t", t=nk),
                            blob_t[:, irc:irc + nk * P].rearrange(
                                "p (n t) -> p n t", t=nk),
                            cwv_t[:, t - nk:t, None]
                            .rearrange("p t o -> p o t")
                            .broadcast_to([P, P, nk]),
                            op=mybir.AluOpType.is_equal,
                        )
                        schunks.append((kdone, nk, s4))
                        kdone += nk
                    g8 = gg.tile([P, RGRP * HID], BF, tag="G")
                    nc.scalar.activation(
                        g8[:, 0:nr * HID], h8_ps[:, 0:nr * HID],
                        mybir.ActivationFunctionType.Relu,
                    )
                    for k0, nk, s4 in schunks:
                        s4v = s4[:, 0:nk * P].rearrange(
                            "p (n t) -> p n t", t=nk)
                        for i in range(nk):
                            ti = rdone + k0 + i
                            nc.tensor.matmul(
                                ptq[:, jr * P:(jr + 1) * P],
                                lhsT=g8[:, (k0 + i) * HID:(k0 + i + 1) * HID],
                                rhs=s4v[:, :, i],
                                start=(ti == 0), stop=(ti == tj - 1),
                            )
                    rdone += nr
                # quad boundary: run update MLP on 4 windows at once
                if jr == qw - 1:
                    w = qw * P
                    n0 = q * QUAD * P
                    hh4 = gn.tile([HID, QUAD * P], BF, tag="hh")
                    if CFG["hh_act"]:
                        nc.scalar.copy(hh4[:, 0:w], ptq[:, 0:w])
                    else:
                        nc.vector.tensor_copy(hh4[:, 0:w], ptq[:, 0:w])
                    h2_ps = p2.tile([HID, QUAD * P], F32, space="PSUM")
                    nc.tensor.matmul(
                        h2_ps[:, 0:w], lhsT=uw1_t,
                        rhs=xcon_t[:, n0:n0 + w],
                        start=True, stop=False,
                    )
                    nc.tensor.matmul(
                        h2_ps[:, 0:w], lhsT=wc_t, rhs=hh4[:, 0:w],
                        start=False, stop=True,
                    )
                    h2r = gn.tile([HID, QUAD * P], BF, tag="h2r")
                    if CFG["h2r_act"]:
                        nc.scalar.activation(
                            h2r[:, 0:w], h2_ps[:, 0:w],
                            mybir.ActivationFunctionType.Relu,
                        )
                    else:
                        nc.vector.tensor_scalar_max(
                            h2r[:, 0:w], h2_ps[:, 0:w], 0.0)
                    o_ps = po.tile([64, QUAD * P], F32, space="PSUM")
                    nc.tensor.matmul(o_ps[:, 0:w], lhsT=uw2_t,
                                     rhs=h2r[:, 0:w], start=True, stop=True)
                    qo = q % OGRP
                    if qo == 0:
                        o_sb = go.tile([64, OGRP * QUAD * P], BF, tag="o")
                    nc.vector.tensor_copy(
                        o_sb[:, qo * QUAD * P:qo * QUAD * P + w],
                        o_ps[:, 0:w])
                    if qo == OGRP - 1 or j == nslots - 1:
                        ow = (q - qo) * QUAD * P
                        nc.gpsimd.dma_start(
                            out_d[:, ow:n0 + w],
                            o_sb[:, 0:n0 + w - ow],
                        )
    nc.finalize()
    return nc


def _schedule(col, n_nodes):
    """Assign 128-node blocks to (core, slot) and derive the shared
    per-slot tile schedule."""
    nblk = -(-n_nodes // P)
    nslots = -(-nblk // NCORES)
    nblk_pad = nslots * NCORES
    nsh = nslots * P

    blk = (col >> 7).astype(np.int64)
    order = np.argsort(blk, kind="stable")
    bc = np.bincount(blk, minlength=nblk_pad)
    bstart = np.zeros(nblk_pad + 1, np.int64)
    np.cumsum(bc, out=bstart[1:])

    sorted_blocks = np.argsort(-bc, kind="stable")
    blk_assign = sorted_blocks.reshape(nslots, NCORES)   # [slot, core]
    grp_max = bc[blk_assign].max(axis=1)
    t_sched = [int(v) for v in np.maximum(1, -(-grp_max // P))]
    return t_sched, blk_assign, order, bc, bstart, nslots, nsh


def kernel(x, edge_index, edge_attr, u, node_batch, wts,
           mw1, mb1, mw2, mb2, uw1, ub1, uw2, ub2):
    x = np.asarray(x, np.float32)
    edge_index = np.asarray(edge_index)
    edge_attr = np.asarray(edge_attr, np.float32)
    u = np.asarray(u, np.float32)
    node_batch = np.asarray(node_batch).astype(np.int64)
    wts = np.asarray(wts, np.float32).reshape(-1)
    mw1 = np.asarray(mw1, np.float32)
    mb1 = np.asarray(mb1, np.float32)
    mw2 = np.asarray(mw2, np.float32)
    mb2 = np.asarray(mb2, np.float32)
    uw1 = np.asarray(uw1, np.float32)
    ub1 = np.asarray(ub1, np.float32)
    uw2 = np.asarray(uw2, np.float32)
    ub2 = np.asarray(ub2, np.float32)

    n_nodes = x.shape[0]
    row = np.asarray(edge_index[0], np.int64)
    col = np.asarray(edge_index[1], np.int64)

    sched = _schedule(col, n_nodes)
    (t_sched, blk_assign, order, bc, bstart, nslots, nsh) = sched
    nt = sum(t_sched)
    e_pad = nt * P

    # per-node stats (host): count, 1/max(cnt,1), weight-sum
    cnt = np.bincount(col, minlength=n_nodes).astype(np.float32)
    rc = 1.0 / np.maximum(cnt, 1.0)
    wsum = np.bincount(col, weights=wts, minlength=n_nodes).astype(np.float32)
    s_node = wsum * rc

    # per-edge
    colof = (col & 127).astype(np.float32)
    wrc = wts * rc[col]

    key = tuple(t_sched)
    if key not in _program_cache:
        _program_cache[key] = _build_program(t_sched)
    nc = _program_cache[key]

    # consts (shared by all cores)
    v_row = mb2 @ uw1[NODE_D:2 * NODE_D, :]              # [HID]
    wc = mw2 @ uw1[NODE_D:2 * NODE_D, :]                 # [HID, HID]
    blob = np.zeros((P, BLOB_W), np.float32)
    blob[0:NODE_D + EDGE_D, _B_MW1:_B_MW1 + HID] = mw1
    blob[NODE_D + EDGE_D, _B_MW1:_B_MW1 + HID] = mb1
    blob[0:NODE_D, _B_UW1:_B_UW1 + HID] = uw1[0:NODE_D, :]
    blob[NODE_D:NODE_D + GLOB_D, _B_UW1:_B_UW1 + HID] = uw1[2 * NODE_D:, :]
    blob[NODE_D + GLOB_D, _B_UW1:_B_UW1 + HID] = v_row
    blob[NODE_D + GLOB_D + 1, _B_UW1:_B_UW1 + HID] = ub1
    blob[0:HID, _B_WC:_B_WC + HID] = wc
    blob[0:HID, _B_UW2:_B_UW2 + 64] = uw2
    for nk in range(1, 5):
        c0 = _B_IR[nk - 1]
        blob[:, c0:c0 + nk * P] = (
            np.arange(nk * P, dtype=np.float32) // nk)[None, :]
    blob_bf = blob.astype(bf16)

    u_per_node = u[node_batch]                           # [N, GLOB_D]

    slot_off = np.zeros(nslots + 1, np.int64)
    np.cumsum(np.asarray(t_sched) * P, out=slot_off[1:])

    in_maps = []
    node_idx_cores = []
    for c in range(NCORES):
        eidx = np.full(e_pad, -1, np.int64)
        nidx = np.full(nsh, -1, np.int64)
        for j in range(nslots):
            b = int(blk_assign[j, c])
            m = int(bc[b])
            o = slot_off[j]
            eidx[o:o + m] = order[bstart[b]:bstart[b] + m]
            n0 = b * P
            nn = min(P, n_nodes - n0)
            if nn > 0:
                nidx[j * P:j * P + nn] = np.arange(n0, n0 + nn)
        evalid = eidx >= 0
        eidxc = np.where(evalid, eidx, 0)
        # eax: [x[row] | edge_attr | 1] * wrc transposed, zeros on pads
        eax = np.empty((e_pad, FEAT), np.float32)
        eax[:, 0:NODE_D] = x[row[eidxc]]
        eax[:, NODE_D:NODE_D + EDGE_D] = edge_attr[eidxc]
        eax[:, NODE_D + EDGE_D] = 1.0
        wcol = np.where(evalid, wrc[eidxc], 0.0).astype(np.float32)
        eax *= wcol[:, None]
        cwv = np.full(e_pad, -1.0, np.float32)
        cwv[evalid] = colof[eidxc[evalid]]

        nvalid = nidx >= 0
        nidxc = np.where(nvalid, nidx, 0)
        xcon = np.zeros((nsh, XCON_R), np.float32)
        xcon[:, 0:NODE_D] = x[nidxc]
        xcon[:, NODE_D:NODE_D + GLOB_D] = u_per_node[nidxc]
        xcon[:, NODE_D + GLOB_D] = s_node[nidxc]
        xcon[:, NODE_D + GLOB_D + 1] = 1.0
        xcon[~nvalid] = 0.0

        in_maps.append({
            "eax": np.ascontiguousarray(eax.T).astype(bf16),
            "cwv": np.ascontiguousarray(
                cwv.reshape(nt, P).T).astype(bf16),
            "blob": blob_bf,
            "xcon": np.ascontiguousarray(xcon.T).astype(bf16),
        })
        node_idx_cores.append((nidx, nvalid))

    res = run_bass_kernel_spmd(nc, in_maps, core_ids=list(range(NCORES)))
    global _last_results
    _last_results = res

    out_full = np.zeros((n_nodes, 64), np.float32)
    for c in range(NCORES):
        nidx, nvalid = node_idx_cores[c]
        oc = np.asarray(res.results[c]["out"], np.float32)   # [64, nsh]
        out_full[nidx[nvalid]] = oc.T[nvalid]
    out_full += ub2[None, :]
    return out_full


# revision 11
# speedup vs baseline: 2.0227x; 1.5833x over previous
"""GNN message-passing kernel for Trainium2 (8 NeuronCores, SPMD).

Strategy (edge-parallel by destination):
  * Host sorts edges by destination node, assigns 128-node blocks to
    (core, window-slot) pairs so per-slot edge counts are balanced across
    cores (one shared compile-time tile schedule for all 8 cores).
  * Host pre-gathers x[row] / edge_attr per edge, folds the per-edge
    scale wrc = wts / max(count[col], 1) directly into the gathered
    features (relu(w*h) == w*relu(h) for w >= 0), and folds mw2 @ uw1r
    into a single weight Wc so the hid-basis scatter feeds the update
    MLP without a per-window mw2 matmul. ub2 is added on the host.
  * All heavy DMA goes through the gpsimd software DGE so packets spread
    across all 16 DMA engines (HWDGE queues pin to a single engine).
    cw is SBUF-resident; eax streams in large groups; xcon streams one
    chunk per window-quad so it never competes with eax at startup.
  * Device, per 128-edge tile: h = eax.T @ mw1_aug (PSUM), relu on ACT
    (one per 8 tiles), one-hot S built on DVE one instr per 4 tiles in
    an interleaved [node, tile] layout (keeps operands packed for DVE
    fast modes), scatter T_w[hid, node] += G.T @ S per 128-node window.
  * Update MLP per 4-window quad: h2 = uw1aug.T @ xcon + Wc.T @ T,
    relu, out = uw2.T @ h2r, written bf16.
  * All matmuls bf16: measured on TRN2, fp8 matmuls are not faster per
    column than bf16 (and fp8 DoubleRow is slower), so bf16 minimizes
    PE time while halving DMA vs f32.
"""
import numpy as np
import ml_dtypes

import concourse.bacc as bacc
import concourse.tile as tile
from concourse import mybir
from concourse.bass_utils import run_bass_kernel_spmd

BF = mybir.dt.bfloat16
F32 = mybir.dt.float32
bf16 = ml_dtypes.bfloat16

P = 128
NCORES = 8
HID = 128
NODE_D = 64
EDGE_D = 32
GLOB_D = 32
FEAT = 98                    # x*w | ea*w | w | zero-pad
XCON_R = 98                  # x | u | s | ones
GROUP = 64                   # 128-edge tiles per eax DMA group
SGRP = 4                     # tiles per S-build batch
RGRP = 4                     # tiles per relu batch (PSUM group)
QUAD = 4                     # windows per update-MLP batch
OGRP = 2                     # quads per output DMA

# const blob column layout (bf16)
_B_MW1 = 0                   # [0:97, 0:128]      mw1_aug (w/ mb1 row)
_B_UW1 = 128                 # [0:98, 128:256]    uw1aug (x|u|v_row|ub1)
_B_WC = 256                  # [0:128, 256:384]   Wc = mw2 @ uw1r
_B_UW2 = 384                 # [0:128, 384:448]   uw2
_B_IR = (448, 576, 832, 1216)  # iotaRep nk=1..4: col c -> c // nk
BLOB_W = 1728

CFG = {
    "geax": 3, "gs": 3, "gg": 3, "gn": 2, "go": 2,
    "ph": 2, "pt": 2, "p2": 2,
    "h2r_act": True,         # h2r relu on ACT (else DVE)
    "hh_act": False,         # hh copy on ACT (else DVE)
}

_program_cache: dict = {}
_last_results = None


def _build_program(t_sched):
    nt = sum(t_sched)
    e_pad = nt * P
    nslots = len(t_sched)
    nsh = nslots * P
    nquads = -(-nslots // QUAD)

    nc = bacc.Bacc()
    eax_d = nc.dram_tensor("eax", [FEAT, e_pad], BF, kind="ExternalInput")
    cwv_d = nc.dram_tensor("cwv", [P, nt], BF, kind="ExternalInput")
    blob_d = nc.dram_tensor("blob", [P, BLOB_W], BF, kind="ExternalInput")
    ub2_d = nc.dram_tensor("ub2", [64, 1], F32, kind="ExternalInput")
    xcon_d = nc.dram_tensor("xcon", [XCON_R, nsh], BF, kind="ExternalInput")
    out_d = nc.dram_tensor("out", [64, nsh], BF, kind="ExternalOutput")

    with tile.TileContext(nc) as tc:
        with (
            tc.tile_pool(name="consts", bufs=1) as consts,
            tc.tile_pool(name="geax", bufs=CFG["geax"]) as geax,
            tc.tile_pool(name="gs", bufs=CFG["gs"]) as gs,
            tc.tile_pool(name="gg", bufs=CFG["gg"]) as gg,
            tc.tile_pool(name="gn", bufs=CFG["gn"]) as gn,
            tc.tile_pool(name="go", bufs=CFG["go"]) as go,
            tc.tile_pool(name="ph", bufs=CFG["ph"], space="PSUM") as ph,
            tc.tile_pool(name="pt", bufs=CFG["pt"], space="PSUM") as pt,
            tc.tile_pool(name="p2", bufs=CFG["p2"], space="PSUM") as p2,
            tc.tile_pool(name="po", bufs=1, space="PSUM") as po,
        ):
            blob_t = consts.tile([P, BLOB_W], BF)
            nc.gpsimd.dma_start(blob_t[:], blob_d[:])
            mw1_t = blob_t[0:FEAT, _B_MW1:_B_MW1 + HID]
            uw1_t = blob_t[0:XCON_R, _B_UW1:_B_UW1 + HID]
            wc_t = blob_t[0:HID, _B_WC:_B_WC + HID]
            uw2_t = blob_t[0:HID, _B_UW2:_B_UW2 + 64]
            ub2_t = consts.tile([64, 1], F32)
            nc.gpsimd.dma_start(ub2_t[:], ub2_d[:])
            cwv_t = consts.tile([P, nt], BF)
            nc.gpsimd.dma_start(cwv_t[:], cwv_d[:])
            xcon_t = consts.tile([XCON_R, nsh], BF)
            nc.gpsimd.dma_start(xcon_t[:], xcon_d[:])

            eax_g = None
            ptq = None
            o_sb = None
            t = 0
            for j in range(nslots):
                tj = t_sched[j]
                q, jr = divmod(j, QUAD)
                if jr == 0:
                    qw = min(QUAD, nslots - q * QUAD)
                    ptq = pt.tile([P, QUAD * P], F32, space="PSUM")
                rdone = 0
                while rdone < tj:
                    nr = min(RGRP, tj - rdone)
                    h8_ps = ph.tile([P, RGRP * HID], F32, space="PSUM")
                    schunks = []
                    kdone = 0
                    while kdone < nr:
                        nk = min(SGRP, nr - kdone)
                        for i in range(kdone, kdone + nk):
                            g, r = divmod(t, GROUP)
                            if r == 0:
                                n = min(GROUP, nt - g * GROUP)
                                eax_g = geax.tile(
                                    [FEAT, GROUP * P], BF, tag="eax")
                                nc.gpsimd.dma_start(
                                    eax_g[:, 0:n * P],
                                    eax_d[:, g * GROUP * P:
                                          (g * GROUP + n) * P],
                                )
                            nc.tensor.matmul(
                                h8_ps[:, i * HID:(i + 1) * HID],
                                lhsT=eax_g[:, r * P:(r + 1) * P],
                                rhs=mw1_t,
                                start=True, stop=True,
                            )
                            t += 1
                        s4 = gs.tile([P, SGRP * P], BF, tag="S")
                        irc = _B_IR[nk - 1]
                        nc.vector.tensor_tensor(
                            s4[:, 0:nk * P].rearrange(
                                "p (n t) -> p n t", t=nk),
                            blob_t[:, irc:irc + nk * P].rearrange(
                                "p (n t) -> p n t", t=nk),
                            cwv_t[:, t - nk:t, None]
                            .rearrange("p t o -> p o t")
                            .broadcast_to([P, P, nk]),
                            op=mybir.AluOpType.is_equal,
                        )
                        schunks.append((kdone, nk, s4))
                        kdone += nk
                    g8 = gg.tile([P, RGRP * HID], BF, tag="G")
                    nc.scalar.activation(
                        g8[:, 0:nr * HID], h8_ps[:, 0:nr * HID],
                        mybir.ActivationFunctionType.Relu,
                    )
                    for k0, nk, s4 in schunks:
                        s4v = s4[:, 0:nk * P].rearrange(
                            "p (n t) -> p n t", t=nk)
                        for i in range(nk):
                            ti = rdone + k0 + i
                            nc.tensor.matmul(
                                ptq[:, jr * P:(jr + 1) * P],
                                lhsT=g8[:, (k0 + i) * HID:(k0 + i + 1) * HID],
                                rhs=s4v[:, :, i],
                                start=(ti == 0), stop=(ti == tj - 1),
                            )
                    rdone += nr
                # quad boundary: run update MLP on 4 windows at once
                if jr == qw - 1:
                    w = qw * P
                    n0 = q * QUAD * P
                    hh4 = gn.tile([HID, QUAD * P], BF, tag="hh")
                    if CFG["hh_act"]:
                        nc.scalar.copy(hh4[:, 0:w], ptq[:, 0:w])
                    else:
                        nc.vector.tensor_copy(hh4[:, 0:w], ptq[:, 0:w])
                    h2_ps = p2.tile([HID, QUAD * P], F32, space="PSUM")
                    nc.tensor.matmul(
                        h2_ps[:, 0:w], lhsT=uw1_t,
                        rhs=xcon_t[:, n0:n0 + w],
                        start=True, stop=False,
                    )
                    nc.tensor.matmul(
                        h2_ps[:, 0:w], lhsT=wc_t, rhs=hh4[:, 0:w],
                        start=False, stop=True,
                    )
                    h2r = gn.tile([HID, QUAD * P], BF, tag="h2r")
                    if CFG["h2r_act"]:
                        nc.scalar.activation(
                            h2r[:, 0:w], h2_ps[:, 0:w],
                            mybir.ActivationFunctionType.Relu,
                        )
                    else:
                        nc.vector.tensor_scalar_max(
                            h2r[:, 0:w], h2_ps[:, 0:w], 0.0)
                    o_ps = po.tile([64, QUAD * P], F32, space="PSUM")
                    nc.tensor.matmul(o_ps[:, 0:w], lhsT=uw2_t,
                                     rhs=h2r[:, 0:w], start=True, stop=True)
                    qo = q % OGRP
                    if qo == 0:
                        o_sb = go.tile([64, OGRP * QUAD * P], BF, tag="o")
                    nc.vector.tensor_scalar(
                        out=o_sb[:, qo * QUAD * P:qo * QUAD * P + w],
                        in0=o_ps[:, 0:w],
                        scalar1=ub2_t[:, 0:1], scalar2=None,
                        op0=mybir.AluOpType.add,
                    )
                    if qo == OGRP - 1 or j == nslots - 1:
                        ow = (q - qo) * QUAD * P
                        nc.gpsimd.dma_start(
                            out_d[:, ow:n0 + w],
                            o_sb[:, 0:n0 + w - ow],
                        )
    nc.finalize()
    return nc


def _schedule(col, n_nodes):
    """Assign 128-node blocks to (core, slot) and derive the shared
    per-slot tile schedule."""
    nblk = -(-n_nodes // P)
    nslots = -(-nblk // NCORES)
    nblk_pad = nslots * NCORES
    nsh = nslots * P

    blk = (col >> 7).astype(np.int64)
    order = np.argsort(blk, kind="stable")
    bc = np.bincount(blk, minlength=nblk_pad)
    bstart = np.zeros(nblk_pad + 1, np.int64)
    np.cumsum(bc, out=bstart[1:])

    sorted_blocks = np.argsort(-bc, kind="stable")
    blk_assign = sorted_blocks.reshape(nslots, NCORES)   # [slot, core]
    grp_max = bc[blk_assign].max(axis=1)
    t_sched = [int(v) for v in np.maximum(1, -(-grp_max // P))]
    return t_sched, blk_assign, order, bc, bstart, nslots, nsh


def kernel(x, edge_index, edge_attr, u, node_batch, wts,
           mw1, mb1, mw2, mb2, uw1, ub1, uw2, ub2):
    x = np.asarray(x, np.float32)
    edge_index = np.asarray(edge_index)
    edge_attr = np.asarray(edge_attr, np.float32)
    u = np.asarray(u, np.float32)
    node_batch = np.asarray(node_batch).astype(np.int64)
    wts = np.asarray(wts, np.float32).reshape(-1)
    mw1 = np.asarray(mw1, np.float32)
    mb1 = np.asarray(mb1, np.float32)
    mw2 = np.asarray(mw2, np.float32)
    mb2 = np.asarray(mb2, np.float32)
    uw1 = np.asarray(uw1, np.float32)
    ub1 = np.asarray(ub1, np.float32)
    uw2 = np.asarray(uw2, np.float32)
    ub2 = np.asarray(ub2, np.float32)

    n_nodes = x.shape[0]
    row = np.asarray(edge_index[0], np.int64)
    col = np.asarray(edge_index[1], np.int64)

    sched = _schedule(col, n_nodes)
    (t_sched, blk_assign, order, bc, bstart, nslots, nsh) = sched
    nt = sum(t_sched)
    e_pad = nt * P

    # per-node stats (host): count, 1/max(cnt,1), weight-sum
    cnt = np.bincount(col, minlength=n_nodes).astype(np.float32)
    rc = 1.0 / np.maximum(cnt, 1.0)
    wsum = np.bincount(col, weights=wts, minlength=n_nodes).astype(np.float32)
    s_node = wsum * rc

    # per-edge
    colof = (col & 127).astype(np.float32)
    wrc = wts * rc[col]

    key = tuple(t_sched)
    if key not in _program_cache:
        _program_cache[key] = _build_program(t_sched)
    nc = _program_cache[key]

    # consts (shared by all cores)
    v_row = mb2 @ uw1[NODE_D:2 * NODE_D, :]              # [HID]
    wc = mw2 @ uw1[NODE_D:2 * NODE_D, :]                 # [HID, HID]
    blob = np.zeros((P, BLOB_W), np.float32)
    blob[0:NODE_D + EDGE_D, _B_MW1:_B_MW1 + HID] = mw1
    blob[NODE_D + EDGE_D, _B_MW1:_B_MW1 + HID] = mb1
    blob[0:NODE_D, _B_UW1:_B_UW1 + HID] = uw1[0:NODE_D, :]
    blob[NODE_D:NODE_D + GLOB_D, _B_UW1:_B_UW1 + HID] = uw1[2 * NODE_D:, :]
    blob[NODE_D + GLOB_D, _B_UW1:_B_UW1 + HID] = v_row
    blob[NODE_D + GLOB_D + 1, _B_UW1:_B_UW1 + HID] = ub1
    blob[0:HID, _B_WC:_B_WC + HID] = wc
    blob[0:HID, _B_UW2:_B_UW2 + 64] = uw2
    for nk in range(1, 5):
        c0 = _B_IR[nk - 1]
        blob[:, c0:c0 + nk * P] = (
            np.arange(nk * P, dtype=np.float32) // nk)[None, :]
    blob_bf = blob.astype(bf16)
    ub2_a = ub2.reshape(64, 1).astype(np.float32)

    u_per_node = u[node_batch]                           # [N, GLOB_D]

    slot_off = np.zeros(nslots + 1, np.int64)
    np.cumsum(np.asarray(t_sched) * P, out=slot_off[1:])

    in_maps = []
    node_idx_cores = []
    for c in range(NCORES):
        eidx = np.full(e_pad, -1, np.int64)
        nidx = np.full(nsh, -1, np.int64)
        for j in range(nslots):
            b = int(blk_assign[j, c])
            m = int(bc[b])
            o = slot_off[j]
            eidx[o:o + m] = order[bstart[b]:bstart[b] + m]
            n0 = b * P
            nn = min(P, n_nodes - n0)
            if nn > 0:
                nidx[j * P:j * P + nn] = np.arange(n0, n0 + nn)
        evalid = eidx >= 0
        eidxc = np.where(evalid, eidx, 0)
        # eax: [x[row] | edge_attr | 1] * wrc transposed, zeros on pads
        eax = np.empty((e_pad, FEAT), np.float32)
        eax[:, 0:NODE_D] = x[row[eidxc]]
        eax[:, NODE_D:NODE_D + EDGE_D] = edge_attr[eidxc]
        eax[:, NODE_D + EDGE_D] = 1.0
        eax[:, FEAT - 1] = 0.0
        wcol = np.where(evalid, wrc[eidxc], 0.0).astype(np.float32)
        eax *= wcol[:, None]
        cwv = np.full(e_pad, -1.0, np.float32)
        cwv[evalid] = colof[eidxc[evalid]]

        nvalid = nidx >= 0
        nidxc = np.where(nvalid, nidx, 0)
        xcon = np.zeros((nsh, XCON_R), np.float32)
        xcon[:, 0:NODE_D] = x[nidxc]
        xcon[:, NODE_D:NODE_D + GLOB_D] = u_per_node[nidxc]
        xcon[:, NODE_D + GLOB_D] = s_node[nidxc]
        xcon[:, NODE_D + GLOB_D + 1] = 1.0
        xcon[~nvalid] = 0.0

        in_maps.append({
            "eax": np.ascontiguousarray(eax.T).astype(bf16),
            "cwv": np.ascontiguousarray(
                cwv.reshape(nt, P).T).astype(bf16),
            "blob": blob_bf,
            "ub2": ub2_a,
            "xcon": np.ascontiguousarray(xcon.T).astype(bf16),
        })
        node_idx_cores.append((nidx, nvalid))

    res = run_bass_kernel_spmd(nc, in_maps, core_ids=list(range(NCORES)))
    global _last_results
    _last_results = res

    out_full = np.zeros((n_nodes, 64), np.float32)
    for c in range(NCORES):
        nidx, nvalid = node_idx_cores[c]
        oc = np.asarray(res.results[c]["out"], np.float32)   # [64, nsh]
        out_full[nidx[nvalid]] = oc.T[nvalid]
    return out_full


# revision 16
# speedup vs baseline: 2.1437x; 1.0598x over previous
"""GNN message-passing kernel for Trainium2 (8 NeuronCores, SPMD).

Strategy (edge-parallel by destination):
  * Host sorts edges by destination node, assigns 128-node blocks to
    (core, window-slot) pairs so per-slot edge counts are balanced across
    cores (one shared compile-time tile schedule for all 8 cores).
  * Host pre-gathers x[row] / edge_attr per edge, folds the per-edge
    scale wrc = wts / max(count[col], 1) directly into the gathered
    features (relu(w*h) == w*relu(h) for w >= 0), and folds mw2 @ uw1r
    into a single weight Wc so the hid-basis scatter feeds the update
    MLP without a per-window mw2 matmul. ub2 is added on the host.
  * All heavy DMA goes through the gpsimd software DGE so packets spread
    across all 16 DMA engines (HWDGE queues pin to a single engine).
    cw is SBUF-resident; eax streams in large groups; xcon streams one
    chunk per window-quad so it never competes with eax at startup.
  * Device, per 128-edge tile: h = eax.T @ mw1_aug (PSUM), relu on ACT
    (one per 8 tiles), one-hot S built on DVE one instr per 4 tiles in
    an interleaved [node, tile] layout (keeps operands packed for DVE
    fast modes), scatter T_w[hid, node] += G.T @ S per 128-node window.
  * Update MLP per 4-window quad: h2 = uw1aug.T @ xcon + Wc.T @ T,
    relu, out = uw2.T @ h2r, written bf16.
  * All matmuls bf16: measured on TRN2, fp8 matmuls are not faster per
    column than bf16 (and fp8 DoubleRow is slower), so bf16 minimizes
    PE time while halving DMA vs f32.
"""
import numpy as np
import ml_dtypes

import concourse.bacc as bacc
import concourse.tile as tile
from concourse import mybir
from concourse.bass_utils import run_bass_kernel_spmd

BF = mybir.dt.bfloat16
F32 = mybir.dt.float32
bf16 = ml_dtypes.bfloat16

P = 128
NCORES = 8
HID = 128
NODE_D = 64
EDGE_D = 32
GLOB_D = 32
FEAT = 98                    # x*w | ea*w | w | zero-pad
XCON_R = 98                  # x | u | s | ones
GROUP = 64                   # 128-edge tiles per eax DMA group
SGRP = 4                     # tiles per S-build batch
RGRP = 4                     # tiles per relu batch (PSUM group)
QUAD = 4                     # windows per update-MLP batch
OGRP = 2                     # quads per output DMA

# const blob column layout (bf16)
_B_MW1 = 0                   # [0:97, 0:128]      mw1_aug (w/ mb1 row)
_B_UW1 = 128                 # [0:98, 128:256]    uw1aug (x|u|v_row|ub1)
_B_WC = 256                  # [0:128, 256:384]   Wc = mw2 @ uw1r
_B_UW2 = 384                 # [0:128, 384:448]   uw2
_B_IR = (448, 576, 832, 1216)  # iotaRep nk=1..4: col c -> c // nk
BLOB_W = 1728

CFG = {
    "geax": 3, "gs": 3, "gg": 3, "gn": 2, "go": 2,
    "ph": 2, "pt": 2, "p2": 2,
    "h2r_act": True,         # h2r relu on ACT (else DVE)
    "hh_act": False,         # hh copy on ACT (else DVE)
}

_program_cache: dict = {}
_last_results = None


def _build_program(t_sched, ranges):
    nt = sum(t_sched)
    e_pad = nt * P
    nslots = len(t_sched)
    nsh = nslots * P
    nquads = -(-nslots // QUAD)

    nc = bacc.Bacc()
    eax_d = nc.dram_tensor("eax", [FEAT, e_pad], BF, kind="ExternalInput")
    cwv_d = nc.dram_tensor("cwv", [P, nt], BF, kind="ExternalInput")
    blob_d = nc.dram_tensor("blob", [P, BLOB_W], BF, kind="ExternalInput")
    ub2_d = nc.dram_tensor("ub2", [64, 1], F32, kind="ExternalInput")
    xcon_d = nc.dram_tensor("xcon", [XCON_R, nsh], BF, kind="ExternalInput")
    out_d = nc.dram_tensor("out", [64, nsh], BF, kind="ExternalOutput")

    with tile.TileContext(nc) as tc:
        with (
            tc.tile_pool(name="consts", bufs=1) as consts,
            tc.tile_pool(name="geax", bufs=CFG["geax"]) as geax,
            tc.tile_pool(name="gs", bufs=CFG["gs"]) as gs,
            tc.tile_pool(name="gg", bufs=CFG["gg"]) as gg,
            tc.tile_pool(name="gn", bufs=CFG["gn"]) as gn,
            tc.tile_pool(name="go", bufs=CFG["go"]) as go,
            tc.tile_pool(name="ph", bufs=CFG["ph"], space="PSUM") as ph,
            tc.tile_pool(name="pt", bufs=CFG["pt"], space="PSUM") as pt,
            tc.tile_pool(name="p2", bufs=CFG["p2"], space="PSUM") as p2,
            tc.tile_pool(name="po", bufs=1, space="PSUM") as po,
        ):
            blob_t = consts.tile([P, BLOB_W], BF)
            nc.gpsimd.dma_start(blob_t[:], blob_d[:])
            mw1_t = blob_t[0:FEAT, _B_MW1:_B_MW1 + HID]
            uw1_t = blob_t[0:XCON_R, _B_UW1:_B_UW1 + HID]
            wc_t = blob_t[0:HID, _B_WC:_B_WC + HID]
            uw2_t = blob_t[0:HID, _B_UW2:_B_UW2 + 64]
            ub2_t = consts.tile([64, 1], F32)
            nc.gpsimd.dma_start(ub2_t[:], ub2_d[:])
            cwv_t = consts.tile([P, nt], BF)
            nc.gpsimd.dma_start(cwv_t[:], cwv_d[:])
            xcon_t = consts.tile([XCON_R, nsh], BF)
            nc.gpsimd.dma_start(xcon_t[:], xcon_d[:])

            eax_g = None
            ptq = None
            o_sb = None
            t = 0
            for j in range(nslots):
                tj = t_sched[j]
                q, jr = divmod(j, QUAD)
                if jr == 0:
                    qw = min(QUAD, nslots - q * QUAD)
                    ptq = pt.tile([P, QUAD * P], F32, space="PSUM")
                rj = ranges[j]
                rdone = 0
                while rdone < tj:
                    nr = min(RGRP, tj - rdone)
                    h8_ps = ph.tile([P, RGRP * HID], F32, space="PSUM")
                    schunks = []
                    kdone = 0
                    while kdone < nr:
                        nk = min(SGRP, nr - kdone)
                        for i in range(kdone, kdone + nk):
                            g, r = divmod(t, GROUP)
                            if r == 0:
                                n = min(GROUP, nt - g * GROUP)
                                eax_g = geax.tile(
                                    [FEAT, GROUP * P], BF, tag="eax")
                                nc.gpsimd.dma_start(
                                    eax_g[:, 0:n * P],
                                    eax_d[:, g * GROUP * P:
                                          (g * GROUP + n) * P],
                                )
                            nc.tensor.matmul(
                                h8_ps[:, i * HID:(i + 1) * HID],
                                lhsT=eax_g[:, r * P:(r + 1) * P],
                                rhs=mw1_t,
                                start=True, stop=True,
                            )
                            t += 1
                        # shared [clo, chi) col range for this chunk's tiles
                        k0g = rdone + kdone
                        if k0g == 0:
                            clo, chi = 0, P
                        else:
                            clo = min(rj[k][0] for k in range(k0g, k0g + nk))
                            chi = max(rj[k][1] for k in range(k0g, k0g + nk))
                        wd = chi - clo
                        s4 = gs.tile([P, SGRP * P], BF, tag="S")
                        irc = _B_IR[nk - 1]
                        nc.vector.tensor_tensor(
                            s4[:, 0:nk * wd].rearrange(
                                "p (n t) -> p n t", t=nk),
                            blob_t[:, irc + clo * nk:irc + chi * nk]
                            .rearrange("p (n t) -> p n t", t=nk),
                            cwv_t[:, t - nk:t, None]
                            .rearrange("p t o -> p o t")
                            .broadcast_to([P, wd, nk]),
                            op=mybir.AluOpType.is_equal,
                        )
                        schunks.append((kdone, nk, s4, clo, chi))
                        kdone += nk
                    g8 = gg.tile([P, RGRP * HID], BF, tag="G")
                    nc.scalar.activation(
                        g8[:, 0:nr * HID], h8_ps[:, 0:nr * HID],
                        mybir.ActivationFunctionType.Relu,
                    )
                    for k0, nk, s4, clo, chi in schunks:
                        s4v = s4[:, 0:nk * (chi - clo)].rearrange(
                            "p (n t) -> p n t", t=nk)
                        for i in range(nk):
                            ti = rdone + k0 + i
                            if ti == 0:
                                lo, hi = 0, P
                            else:
                                lo, hi = rj[ti]
                            nc.tensor.matmul(
                                ptq[:, jr * P + lo:jr * P + hi],
                                lhsT=g8[:, (k0 + i) * HID:(k0 + i + 1) * HID],
                                rhs=s4v[:, lo - clo:hi - clo, i],
                                start=(ti == 0), stop=(ti == tj - 1),
                                skip_group_check=True,
                            )
                    rdone += nr
                # quad boundary: run update MLP on 4 windows at once
                if jr == qw - 1:
                    w = qw * P
                    n0 = q * QUAD * P
                    hh4 = gn.tile([HID, QUAD * P], BF, tag="hh")
                    if CFG["hh_act"]:
                        nc.scalar.copy(hh4[:, 0:w], ptq[:, 0:w])
                    else:
                        nc.vector.tensor_copy(hh4[:, 0:w], ptq[:, 0:w])
                    h2_ps = p2.tile([HID, QUAD * P], F32, space="PSUM")
                    nc.tensor.matmul(
                        h2_ps[:, 0:w], lhsT=uw1_t,
                        rhs=xcon_t[:, n0:n0 + w],
                        start=True, stop=False,
                    )
                    nc.tensor.matmul(
                        h2_ps[:, 0:w], lhsT=wc_t, rhs=hh4[:, 0:w],
                        start=False, stop=True,
                    )
                    h2r = gn.tile([HID, QUAD * P], BF, tag="h2r")
                    if CFG["h2r_act"]:
                        nc.scalar.activation(
                            h2r[:, 0:w], h2_ps[:, 0:w],
                            mybir.ActivationFunctionType.Relu,
                        )
                    else:
                        nc.vector.tensor_scalar_max(
                            h2r[:, 0:w], h2_ps[:, 0:w], 0.0)
                    o_ps = po.tile([64, QUAD * P], F32, space="PSUM")
                    nc.tensor.matmul(o_ps[:, 0:w], lhsT=uw2_t,
                                     rhs=h2r[:, 0:w], start=True, stop=True)
                    qo = q % OGRP
                    if qo == 0:
                        o_sb = go.tile([64, OGRP * QUAD * P], BF, tag="o")
                    nc.vector.tensor_scalar(
                        out=o_sb[:, qo * QUAD * P:qo * QUAD * P + w],
                        in0=o_ps[:, 0:w],
                        scalar1=ub2_t[:, 0:1], scalar2=None,
                        op0=mybir.AluOpType.add,
                    )
                    if qo == OGRP - 1 or j == nslots - 1:
                        ow = (q - qo) * QUAD * P
                        nc.gpsimd.dma_start(
                            out_d[:, ow:n0 + w],
                            o_sb[:, 0:n0 + w - ow],
                        )
    nc.finalize()
    return nc


def _schedule(col, n_nodes):
    """Assign 128-node blocks to (core, slot) and derive the shared
    per-slot tile schedule."""
    nblk = -(-n_nodes // P)
    nslots = -(-nblk // NCORES)
    nblk_pad = nslots * NCORES
    nsh = nslots * P

    blk = (col >> 7).astype(np.int64)
    order = np.argsort(col, kind="stable")
    bc = np.bincount(blk, minlength=nblk_pad)
    bstart = np.zeros(nblk_pad + 1, np.int64)
    np.cumsum(bc, out=bstart[1:])

    sorted_blocks = np.argsort(-bc, kind="stable")
    blk_assign = sorted_blocks.reshape(nslots, NCORES)   # [slot, core]
    grp_max = bc[blk_assign].max(axis=1)
    t_sched = [int(v) for v in np.maximum(1, -(-grp_max // P))]
    return t_sched, blk_assign, order, bc, bstart, nslots, nsh


def kernel(x, edge_index, edge_attr, u, node_batch, wts,
           mw1, mb1, mw2, mb2, uw1, ub1, uw2, ub2):
    x = np.asarray(x, np.float32)
    edge_index = np.asarray(edge_index)
    edge_attr = np.asarray(edge_attr, np.float32)
    u = np.asarray(u, np.float32)
    node_batch = np.asarray(node_batch).astype(np.int64)
    wts = np.asarray(wts, np.float32).reshape(-1)
    mw1 = np.asarray(mw1, np.float32)
    mb1 = np.asarray(mb1, np.float32)
    mw2 = np.asarray(mw2, np.float32)
    mb2 = np.asarray(mb2, np.float32)
    uw1 = np.asarray(uw1, np.float32)
    ub1 = np.asarray(ub1, np.float32)
    uw2 = np.asarray(uw2, np.float32)
    ub2 = np.asarray(ub2, np.float32)

    n_nodes = x.shape[0]
    row = np.asarray(edge_index[0], np.int64)
    col = np.asarray(edge_index[1], np.int64)

    sched = _schedule(col, n_nodes)
    (t_sched, blk_assign, order, bc, bstart, nslots, nsh) = sched
    nt = sum(t_sched)
    e_pad = nt * P

    # per-node stats (host): count, 1/max(cnt,1), weight-sum
    cnt = np.bincount(col, minlength=n_nodes).astype(np.float32)
    rc = 1.0 / np.maximum(cnt, 1.0)
    wsum = np.bincount(col, weights=wts, minlength=n_nodes).astype(np.float32)
    s_node = wsum * rc

    # per-edge
    colof = (col & 127).astype(np.float32)
    wrc = wts * rc[col]

    slot_off0 = np.zeros(nslots + 1, np.int64)
    np.cumsum(np.asarray(t_sched) * P, out=slot_off0[1:])

    # per-core edge slots + cwv; ranges = per-(slot,tile) union col span
    core_idx = []
    cwv_cores = []
    rlo = np.full(nt, P, np.int64)
    rhi = np.zeros(nt, np.int64)
    for c in range(NCORES):
        eidx = np.full(e_pad, -1, np.int64)
        nidx = np.full(nsh, -1, np.int64)
        for j in range(nslots):
            b = int(blk_assign[j, c])
            m = int(bc[b])
            o = slot_off0[j]
            eidx[o:o + m] = order[bstart[b]:bstart[b] + m]
            n0 = b * P
            nn = min(P, n_nodes - n0)
            if nn > 0:
                nidx[j * P:j * P + nn] = np.arange(n0, n0 + nn)
        evalid = eidx >= 0
        eidxc = np.where(evalid, eidx, 0)
        cwv = np.full(e_pad, -1.0, np.float32)
        cwv[evalid] = colof[eidxc[evalid]]
        cwv_t = cwv.reshape(nt, P)
        vm = cwv_t >= 0
        np.minimum(rlo, np.where(vm, cwv_t, P).min(axis=1).astype(np.int64),
                   out=rlo)
        np.maximum(rhi, (np.where(vm, cwv_t, -1.0).max(axis=1) + 1)
                   .astype(np.int64), out=rhi)
        core_idx.append((eidx, evalid, eidxc, nidx))
        cwv_cores.append(cwv)
    rlo = np.minimum(rlo, rhi)           # empty tiles -> [hi, hi)
    ranges = []
    for j in range(nslots):
        t0 = slot_off0[j] // P
        ranges.append([(int(rlo[t0 + k]), int(rhi[t0 + k]))
                       for k in range(t_sched[j])])

    key = (tuple(t_sched),
           tuple(v for rj in ranges for lh in rj for v in lh))
    if key not in _program_cache:
        _program_cache[key] = _build_program(t_sched, ranges)
    nc = _program_cache[key]

    # consts (shared by all cores)
    v_row = mb2 @ uw1[NODE_D:2 * NODE_D, :]              # [HID]
    wc = mw2 @ uw1[NODE_D:2 * NODE_D, :]                 # [HID, HID]
    blob = np.zeros((P, BLOB_W), np.float32)
    blob[0:NODE_D + EDGE_D, _B_MW1:_B_MW1 + HID] = mw1
    blob[NODE_D + EDGE_D, _B_MW1:_B_MW1 + HID] = mb1
    blob[0:NODE_D, _B_UW1:_B_UW1 + HID] = uw1[0:NODE_D, :]
    blob[NODE_D:NODE_D + GLOB_D, _B_UW1:_B_UW1 + HID] = uw1[2 * NODE_D:, :]
    blob[NODE_D + GLOB_D, _B_UW1:_B_UW1 + HID] = v_row
    blob[NODE_D + GLOB_D + 1, _B_UW1:_B_UW1 + HID] = ub1
    blob[0:HID, _B_WC:_B_WC + HID] = wc
    blob[0:HID, _B_UW2:_B_UW2 + 64] = uw2
    for nk in range(1, 5):
        c0 = _B_IR[nk - 1]
        blob[:, c0:c0 + nk * P] = (
            np.arange(nk * P, dtype=np.float32) // nk)[None, :]
    blob_bf = blob.astype(bf16)
    ub2_a = ub2.reshape(64, 1).astype(np.float32)

    u_per_node = u[node_batch]                           # [N, GLOB_D]

    in_maps = []
    node_idx_cores = []
    for c in range(NCORES):
        eidx, evalid, eidxc, nidx = core_idx[c]
        cwv = cwv_cores[c]
        # eax: [x[row] | edge_attr | 1] * wrc transposed, zeros on pads
        eax = np.empty((e_pad, FEAT), np.float32)
        eax[:, 0:NODE_D] = x[row[eidxc]]
        eax[:, NODE_D:NODE_D + EDGE_D] = edge_attr[eidxc]
        eax[:, NODE_D + EDGE_D] = 1.0
        eax[:, FEAT - 1] = 0.0
        wcol = np.where(evalid, wrc[eidxc], 0.0).astype(np.float32)
        eax *= wcol[:, None]

        nvalid = nidx >= 0
        nidxc = np.where(nvalid, nidx, 0)
        xcon = np.zeros((nsh, XCON_R), np.float32)
        xcon[:, 0:NODE_D] = x[nidxc]
        xcon[:, NODE_D:NODE_D + GLOB_D] = u_per_node[nidxc]
        xcon[:, NODE_D + GLOB_D] = s_node[nidxc]
        xcon[:, NODE_D + GLOB_D + 1] = 1.0
        xcon[~nvalid] = 0.0

        in_maps.append({
            "eax": np.ascontiguousarray(eax.T).astype(bf16),
            "cwv": np.ascontiguousarray(
                cwv.reshape(nt, P).T).astype(bf16),
            "blob": blob_bf,
            "ub2": ub2_a,
            "xcon": np.ascontiguousarray(xcon.T).astype(bf16),
        })
        node_idx_cores.append((nidx, nvalid))

    res = run_bass_kernel_spmd(nc, in_maps, core_ids=list(range(NCORES)))
    global _last_results
    _last_results = res

    out_full = np.zeros((n_nodes, 64), np.float32)
    for c in range(NCORES):
        nidx, nvalid = node_idx_cores[c]
        oc = np.asarray(res.results[c]["out"], np.float32)   # [64, nsh]
        out_full[nidx[nvalid]] = oc.T[nvalid]
    return out_full
